# revision 13
# baseline (speedup 1.0000x reference)
"""Distributed multi-head attention kernel for 8 TRN2 NeuronCores.

Problem: x[4,2048,1024] -> qkv proj (w_qkv[3072,1024]) -> 16-head attention
         -> out proj (w_out[1024,1024], b_out) -> [4,2048,1024]

Sharding: core c handles batch b=c//2 and heads (c%2)*8 .. (c%2)*8+8
(data parallel over batch x tensor parallel over heads). Each pair of
cores {2b, 2b+1} reduce-scatters the output projection partial sums
(split along hidden-out), one collective per 512-token chunk so comm
overlaps the next chunk's compute.

Per-core compute (bf16 matmuls, fp32 PSUM):
  Scores are computed transposed, S^T [j, i], two heads packed in PE
  row groups 0/64. exp runs on ScalarE with the 1/sqrt(d) scale folded
  in; every 6th unit's exp is offloaded to VectorE via the Schraudolph
  bit-trick (int32(x*2^23*log2e + C) reinterpreted as float) to keep
  ScalarE off the critical path. Softmax denominators come from
  ones-matmul column sums packed in PE col groups 0/64 so each head's
  denominator lands replicated under its own output partitions. O^T
  accumulates in PSUM (V stationary, col groups 0/64); VectorE applies
  a fast reciprocal and normalizes into mergedT (bf16), which feeds
  the output projection as the moving operand.

  The attention stream emits scores two (pr, jt)-units ahead so the
  exp pipeline never waits on the PE round trip. PSUM: 2x2 score
  banks + 2 O^T banks + 2 denominator banks = 8; projection/outproj
  accumulators time-share the O^T/denominator pool tags.
"""

import numpy as np
import ml_dtypes

import concourse.bass as bass
import concourse.mybir as mybir
import concourse.tile as tile
from concourse import bacc
from concourse.bass_utils import run_bass_kernel_spmd

B, N, H = 4, 2048, 1024
NH, DH = 16, 64
NCORES = 8
HH = 512          # head dims per core (8 heads x 64)
KH = H // 128     # 8 hidden k-tiles
NJT = N // 128    # 16 token j-tiles
NCK = N // 512    # 4 token chunks
MT = HH // 128    # 4 head-dim partition tiles per core
SCALE = DH ** -0.5

# Schraudolph fast-exp constants (scale folded into C1)
FE_C1 = float(2.0 ** 23 * 1.4426950408889634 * SCALE)
FE_C2 = float(127 * 2 ** 23 - 366000.0)

BF16 = mybir.dt.bfloat16
F32 = mybir.dt.float32
I32 = mybir.dt.int32
Exp = mybir.ActivationFunctionType.Exp
MUL = mybir.AluOpType.mult
ADD = mybir.AluOpType.add

RG = [[0, 1], [2, 3], [4, 5], [6, 7]]

_cache = {}


def _build():
    nc = bacc.Bacc(
        "TRN2", target_bir_lowering=False, debug=False, num_devices=NCORES
    )
    xT = nc.dram_tensor("xT", [H, N], BF16, kind="ExternalInput").ap()
    wqT = nc.dram_tensor("wqT", [H, HH], BF16, kind="ExternalInput").ap()
    wkT = nc.dram_tensor("wkT", [H, HH], BF16, kind="ExternalInput").ap()
    wvT = nc.dram_tensor("wvT", [H, HH], BF16, kind="ExternalInput").ap()
    woT = nc.dram_tensor("woT", [HH, H], BF16, kind="ExternalInput").ap()
    hbT = nc.dram_tensor("hbT", [128, KH], F32, kind="ExternalInput").ap()
    out_e = nc.dram_tensor("out", [H // 2, N], F32, kind="ExternalOutput").ap()

    with tile.TileContext(nc) as tc:
        with (
            tc.tile_pool(name="singles", bufs=1) as singles,
            tc.tile_pool(name="psA", bufs=2, space="PSUM") as psA,
            tc.tile_pool(name="psB", bufs=2, space="PSUM") as psB,
            tc.tile_pool(name="stps", bufs=2, space="PSUM") as stps,
            tc.tile_pool(name="pe", bufs=10) as pe_pool,
            tc.tile_pool(name="pei", bufs=2) as pei_pool,
            tc.tile_pool(name="rsb", bufs=4) as r_pool,
            tc.tile_pool(name="osb", bufs=3) as osb_pool,
            tc.tile_pool(name="dram", bufs=1, space="DRAM") as dram,
        ):
            x_sb = singles.tile([128, KH, N], BF16)
            wq_sb = singles.tile([128, KH, HH], BF16)
            wk_sb = singles.tile([128, KH, HH], BF16)
            wv_sb = singles.tile([128, KH, HH], BF16)
            wo_sb = singles.tile([128, MT, H], BF16)
            hb_sb = singles.tile([128, KH], F32)
            qT_sb = singles.tile([128, MT, N], BF16)
            kT_sb = singles.tile([128, MT, N], BF16)
            v_sb = singles.tile([128, NJT, 8, DH], BF16)
            mT_sb = singles.tile([128, MT, N], BF16)
            ones_sb = singles.tile([128, DH], BF16)

            rs_in = []
            rs_out = []
            for c in range(NCK + 1):
                w = 512 if c < NCK - 1 else 256
                t_in = dram.tile([H, w], F32, tag=f"rsin{c}", name=f"rsin{c}")
                t_out = dram.tile([H // 2, w], F32, tag=f"rsout{c}",
                                  name=f"rsout{c}")
                rs_in.append(t_in)
                rs_out.append(t_out)

            nc.vector.memset(ones_sb, 1.0)
            nc.sync.dma_start(out=hb_sb, in_=hbT)
            for k in range(KH):
                nc.sync.dma_start(out=wk_sb[:, k, :], in_=wkT[k * 128:(k + 1) * 128, :])
                nc.sync.dma_start(out=x_sb[:, k, :], in_=xT[k * 128:(k + 1) * 128, :])
            for k in range(KH):
                nc.sync.dma_start(out=wq_sb[:, k, :], in_=wqT[k * 128:(k + 1) * 128, :])
                nc.sync.dma_start(out=wv_sb[:, k, :], in_=wvT[k * 128:(k + 1) * 128, :])
            for m in range(MT):
                nc.sync.dma_start(out=wo_sb[:, m, :], in_=woT[m * 128:(m + 1) * 128, :])

            def kq_proj(w_sb, dst, m, ci, tg):
                pool = psA if tg == 0 else psB
                ps = pool.tile([128, 512], F32, tag="a" if tg == 0 else "b",
                               name=f"kq{m}_{ci}")
                for k in range(KH):
                    nc.tensor.matmul(
                        ps,
                        lhsT=w_sb[:, k, m * 128:(m + 1) * 128],
                        rhs=x_sb[:, k, ci * 512:(ci + 1) * 512],
                        start=(k == 0), stop=(k == KH - 1),
                    )
                nc.vector.tensor_copy(out=dst[:, m, ci * 512:(ci + 1) * 512], in_=ps)

            def v_proj(jt, tg):
                pool = psA if tg == 0 else psB
                ps = pool.tile([128, 512], F32, tag="a" if tg == 0 else "b",
                               name=f"vp{jt}")
                for k in range(KH):
                    nc.tensor.matmul(
                        ps,
                        lhsT=x_sb[:, k, jt * 128:(jt + 1) * 128],
                        rhs=wv_sb[:, k, :],
                        start=(k == 0), stop=(k == KH - 1),
                    )
                nc.vector.tensor_copy(
                    out=v_sb[:, jt, :, :],
                    in_=ps.rearrange("p (h d) -> p h d", h=8),
                )

            # ---- prologue: K (m0,m1), Q(ci0; m0,m1), V, K (m2,m3), Q rest
            for m in (0, 1):
                for ci in range(NCK):
                    kq_proj(wk_sb, kT_sb, m, ci, ci % 2)
            kq_proj(wq_sb, qT_sb, 0, 0, 0)
            kq_proj(wq_sb, qT_sb, 1, 0, 1)
            for jt in range(NJT):
                v_proj(jt, jt % 2)
            for m in (2, 3):
                for ci in range(NCK):
                    kq_proj(wk_sb, kT_sb, m, ci, ci % 2)
            kq_proj(wq_sb, qT_sb, 2, 0, 0)
            kq_proj(wq_sb, qT_sb, 3, 0, 1)

            def attention_ci(ci):
                units = [(pr, jt) for pr in range(4) for jt in range(NJT)]
                st_tiles = {}

                def emit_scores(u):
                    pr, jt = units[u]
                    st = stps.tile([128, 2, 512], F32, tag="st",
                                   name=f"st{ci}_{u}")
                    nc.tensor.matmul(
                        st[:, 0, :],
                        lhsT=kT_sb[0:64, pr, jt * 128:(jt + 1) * 128],
                        rhs=qT_sb[0:64, pr, ci * 512:(ci + 1) * 512],
                        start=True, stop=True,
                    )
                    nc.tensor.matmul(
                        st[:, 1, :],
                        lhsT=kT_sb[64:128, pr, jt * 128:(jt + 1) * 128],
                        rhs=qT_sb[64:128, pr, ci * 512:(ci + 1) * 512],
                        start=True, stop=True,
                    )
                    st_tiles[u] = st

                def emit_pv(po, pd, pe, pr, jt, first, last):
                    h0, h1 = 2 * pr, 2 * pr + 1
                    nc.tensor.matmul(
                        po[0:64, :], lhsT=v_sb[:, jt, h0, :], rhs=pe[:, 0, :],
                        start=first, stop=last, skip_group_check=True,
                    )
                    nc.tensor.matmul(
                        po[64:128, :], lhsT=v_sb[:, jt, h1, :], rhs=pe[:, 1, :],
                        start=first, stop=last, skip_group_check=True,
                    )
                    nc.tensor.matmul(
                        pd[0:64, :], lhsT=ones_sb, rhs=pe[:, 0, :],
                        start=first, stop=last, skip_group_check=True,
                    )
                    nc.tensor.matmul(
                        pd[64:128, :], lhsT=ones_sb, rhs=pe[:, 1, :],
                        start=first, stop=last, skip_group_check=True,
                    )

                emit_scores(0)
                emit_scores(1)
                po = pd = None
                pending = None
                for u, (pr, jt) in enumerate(units):
                    first, last = (jt == 0), (jt == NJT - 1)
                    if first:
                        po = psA.tile([128, 512], F32, tag="a", name=f"po{pr}")
                        pd = psB.tile([128, 512], F32, tag="b", name=f"pd{pr}")
                    # fast-exp offload to VectorE for every 6th unit (never
                    # the first/last j-tile of a pair: PSUM start/stop bits
                    # must stay temporally ordered when emission is displaced)
                    offload = (u % 6 == 3) and not first and not last
                    pe = pe_pool.tile([128, 2, 512], BF16, tag="pe")
                    st = st_tiles.pop(u)
                    if offload:
                        ti = pei_pool.tile([128, 2, 512], I32, tag="pei")
                        nc.vector.tensor_scalar(
                            out=ti, in0=st, scalar1=FE_C1, scalar2=FE_C2,
                            op0=MUL, op1=ADD,
                        )
                        nc.vector.tensor_copy(out=pe, in_=ti.bitcast(F32))
                    else:
                        nc.scalar.activation(out=pe, in_=st, func=Exp,
                                             scale=SCALE)
                    if u + 2 < len(units):
                        emit_scores(u + 2)
                    if pending is not None:
                        emit_pv(*pending)
                        pending = None
                    if offload:
                        pending = (po, pd, pe, pr, jt, first, last)
                    else:
                        emit_pv(po, pd, pe, pr, jt, first, last)
                    if last:
                        assert pending is None
                        r = r_pool.tile([128, 512], F32, tag="r")
                        nc.vector.reciprocal_approx_fast(out=r, in_=pd)
                        nc.vector.tensor_mul(
                            out=mT_sb[:, pr, ci * 512:(ci + 1) * 512],
                            in0=po, in1=r,
                        )
                        if ci + 1 < NCK:
                            kq_proj(wq_sb, qT_sb, pr, ci + 1, 1)

            def outproj(ci):
                for g in range(2):
                    for m in range(4 * g, 4 * g + 4):
                        pool = psA if m % 2 == 0 else psB
                        pp = pool.tile([128, 512], F32,
                                       tag="a" if m % 2 == 0 else "b",
                                       name=f"pp{ci}_{m}")
                        for kd in range(MT):
                            nc.tensor.matmul(
                                pp,
                                lhsT=wo_sb[:, kd, m * 128:(m + 1) * 128],
                                rhs=mT_sb[:, kd, ci * 512:(ci + 1) * 512],
                                start=(kd == 0), stop=(kd == MT - 1),
                            )
                        ob = osb_pool.tile([128, 512], F32, tag="ob")
                        nc.vector.tensor_scalar_add(
                            out=ob, in0=pp, scalar1=hb_sb[:, m:m + 1]
                        )
                        nc.sync.dma_start(
                            out=rs_in[ci][m * 128:(m + 1) * 128, :], in_=ob
                        )
                    nc.gpsimd.collective_compute(
                        "ReduceScatter",
                        mybir.AluOpType.add,
                        replica_groups=RG,
                        ins=[rs_in[ci][g * 512:(g + 1) * 512, :]],
                        outs=[rs_out[ci][g * 256:(g + 1) * 256, :]],
                    )
                    nc.sync.dma_start(
                        out=out_e[g * 256:(g + 1) * 256, ci * 512:(ci + 1) * 512],
                        in_=rs_out[ci][g * 256:(g + 1) * 256, :],
                    )

            def outproj_last(ci):
                # token-split the final chunk: two half-size collectives,
                # each pipelined behind its own projection slice
                for th in range(2):
                    idx = NCK - 1 + th
                    c0 = ci * 512 + th * 256
                    for m in range(8):
                        pool = psA if m % 2 == 0 else psB
                        pp = pool.tile([128, 512], F32,
                                       tag="a" if m % 2 == 0 else "b",
                                       name=f"pl{th}_{m}")
                        for kd in range(MT):
                            nc.tensor.matmul(
                                pp[:, 0:256],
                                lhsT=wo_sb[:, kd, m * 128:(m + 1) * 128],
                                rhs=mT_sb[:, kd, c0:c0 + 256],
                                start=(kd == 0), stop=(kd == MT - 1),
                            )
                        ob = osb_pool.tile([128, 512], F32, tag="ob")
                        nc.vector.tensor_scalar_add(
                            out=ob[:, 0:256], in0=pp[:, 0:256],
                            scalar1=hb_sb[:, m:m + 1]
                        )
                        nc.sync.dma_start(
                            out=rs_in[idx][m * 128:(m + 1) * 128, :],
                            in_=ob[:, 0:256],
                        )
                    nc.gpsimd.collective_compute(
                        "ReduceScatter",
                        mybir.AluOpType.add,
                        replica_groups=RG,
                        ins=[rs_in[idx][:, :]],
                        outs=[rs_out[idx][:, :]],
                    )
                    nc.sync.dma_start(
                        out=out_e[:, c0:c0 + 256], in_=rs_out[idx][:, :]
                    )

            for ci in range(NCK):
                attention_ci(ci)
                if ci < NCK - 1:
                    outproj(ci)
                else:
                    outproj_last(ci)

    nc.compile()
    return nc


def _get_nc():
    if "nc" not in _cache:
        _cache["nc"] = _build()
    return _cache["nc"]


def _shard_inputs(x, w_qkv, w_out, b_out):
    bf16 = ml_dtypes.bfloat16
    in_maps = []
    for c in range(NCORES):
        b, hh = c // 2, c % 2
        r0 = hh * HH
        hbT = (0.5 * b_out).astype(np.float32).reshape(KH, 128).T
        in_maps.append({
            "xT": np.ascontiguousarray(x[b].T).astype(bf16),
            "wqT": np.ascontiguousarray(w_qkv[r0:r0 + HH, :].T).astype(bf16),
            "wkT": np.ascontiguousarray(w_qkv[H + r0:H + r0 + HH, :].T).astype(bf16),
            "wvT": np.ascontiguousarray(w_qkv[2 * H + r0:2 * H + r0 + HH, :].T).astype(bf16),
            "woT": np.ascontiguousarray(w_out[:, r0:r0 + HH].T).astype(bf16),
            "hbT": np.ascontiguousarray(hbT),
        })
    return in_maps


def _assemble(results):
    out = np.empty((B, N, H), dtype=np.float32)
    t3 = N - 512                       # last chunk: contiguous rank mapping
    for b in range(B):
        lo = np.asarray(results[2 * b]["out"]).astype(np.float32)
        hi = np.asarray(results[2 * b + 1]["out"]).astype(np.float32)
        for g in range(2):
            rs = slice(g * 256, (g + 1) * 256)
            out[b][:t3, g * 512:g * 512 + 256] = lo[rs, :t3].T
            out[b][:t3, g * 512 + 256:(g + 1) * 512] = hi[rs, :t3].T
        out[b][t3:, 0:512] = lo[:, t3:].T
        out[b][t3:, 512:1024] = hi[:, t3:].T
    return out


def run_sharded(x, w_qkv, w_out, b_out, trace=False):
    nc = _get_nc()
    in_maps = _shard_inputs(x, w_qkv, w_out, b_out)
    res = run_bass_kernel_spmd(nc, in_maps, core_ids=list(range(NCORES)),
                               trace=trace)
    return _assemble(res.results), res


def kernel(x, w_qkv, w_out, b_out):
    x = np.asarray(x, dtype=np.float32)
    w_qkv = np.asarray(w_qkv, dtype=np.float32)
    w_out = np.asarray(w_out, dtype=np.float32)
    b_out = np.asarray(b_out, dtype=np.float32)
    out, _ = run_sharded(x, w_qkv, w_out, b_out, trace=False)
    return out
